# revision 29
# baseline (speedup 1.0000x reference)
"""Trainium2 Bass kernel for nn_InternalMAFE_59270548684863.

Structure (final, trace-driven rewrite of the v2 baseline):
  - Only branch 1 (p=7, n=288) of the reference affects the output; the
    n2=1008 branch feeds a dead projection and is never computed.
  - Batch-sharded over 8 cores (512 rows each); softmax normalizes over
    the batch axis, so per-(step, feature) exp-sums are AllReduced.
    Constant-shift softmax exp(s*scale - 50) avoids a cross-core max.
  - Changes vs the v2 baseline:
      * All big tensors are cast to bf16 HOST-side and loaded with plain
        HWDGE DMA (the v2 SWDGE cast-DMA path ran ~3x below line rate);
        the logits weights + first x slots load first so A1 starts ASAP.
      * W_hk = w_k1 @ h1.T is precomputed on the host (removes the
        on-device PE-transpose prep phase).
      * A PE warm-up burst runs at t=0 and the matmul stream is kept
        dense (all logits -> all v -> projection; psL triple-buffered)
        to keep the HAM clock gate at high clock.
      * One combined exp-sum AllReduce [128,21] triggered as soon as the
        den sums retire (exp pair on ACT, den column sums on DVE). The
        ncfw path costs barrier(20-60us)+wakeup(~12us)+mesh(10-28us);
        a raw remote-DMA butterfly was tried and is schedulable via two
        TileContexts with a raw DVE wait between them, but loses: the
        peers' data arrives later than the ncfw protocol completes here.
      * Softmax normalization is fused into the gated scan with one
        scalar_tensor_tensor: ys_i = (t_i * recip) + tanh(..)*sig(..).
      * Output is stored bf16 in one contiguous packed tile (full-width
        evacuations, 4 overlapped store DMAs); host re-packs + up-casts.
"""

import math

import numpy as np
import ml_dtypes

import concourse.bacc as bacc
import concourse.mybir as mybir
import concourse.tile as tile
from concourse.bass_utils import run_bass_kernel_spmd

N_CORES = 8
B = 4096
BL = B // N_CORES  # 512 rows per core
INP = 2016
P1 = 7
N1 = 288
SEQ = 1024
SCALE = 1.0 / math.sqrt(N1)
SHIFT = -50.0
F32 = mybir.dt.float32
BF16 = mybir.dt.bfloat16
AF = mybir.ActivationFunctionType
ALU = mybir.AluOpType
BF16NP = ml_dtypes.bfloat16

NSLOT = 16
CH = [(0, 128), (128, 128), (256, 32)]


def xchunk(i, lt):
    """K-side: (slot, partition offset, count) of x chunk lt of step i."""
    if lt < 2:
        return 2 + 2 * i + lt, 0, 128
    if i < 4:
        return 0, 32 * i, 32
    return 1, 32 * (i - 4), 32


def pack_pos(i):
    """(pack tile index, partition offset) of step i's 32-row tail."""
    if i < 4:
        return 0, 32 * i
    return 1, 32 * (i - 4)


def build():
    nc = bacc.Bacc(
        "TRN2", target_bir_lowering=False, debug=False, num_devices=N_CORES
    )
    # all big inputs pre-cast to bf16 on the host
    xp = nc.dram_tensor("xp", [128, NSLOT * BL], BF16, kind="ExternalInput").ap()
    rkp = nc.dram_tensor("rkp", [128, NSLOT * SEQ], BF16, kind="ExternalInput").ap()
    # wp: [whk lt0|lt1|lt2] [wv lt0|lt1|lt2], each [128, 288]
    wp = nc.dram_tensor("wp", [128, 6 * N1], BF16, kind="ExternalInput").ap()
    plbT = nc.dram_tensor("plbT", [128, 8], F32, kind="ExternalInput").ap()
    gates = nc.dram_tensor("gates", [128, 4], F32, kind="ExternalInput").ap()
    # outT[p, st*BL + b] holds out[b, st*128 + p] (host re-packs)
    outT = nc.dram_tensor("outT", [128, 8 * BL], BF16, kind="ExternalOutput").ap()

    NCOL = 3 * P1  # 21 den columns

    with tile.TileContext(nc) as tc:
        with (
            tc.tile_pool(name="const", bufs=1) as cpool,
            tc.tile_pool(name="data", bufs=1) as dpool,
            tc.tile_pool(name="dram", bufs=1, space="DRAM") as drpool,
        ):
            # ---------- loads (plain HWDGE, bf16) ----------
            # sync ring: logits weights first, then x slots in consumption
            # order (v weights can land later; A2 runs after A1)
            wpb = dpool.tile([128, 6 * N1], BF16, tag="wpb", name="wpb")
            nc.sync.dma_start(wpb[:, 0 : 3 * N1], wp[:, 0 : 3 * N1])
            xpb = dpool.tile([128, NSLOT * BL], BF16, tag="xpb", name="xpb")
            for q in (1, 0, 2):
                cols = slice(q * 2 * BL, (q + 1) * 2 * BL)
                nc.sync.dma_start(xpb[:, cols], xp[:, cols])
            nc.sync.dma_start(wpb[:, 3 * N1 : 6 * N1], wp[:, 3 * N1 : 6 * N1])
            for q in (3, 4, 5, 6, 7):
                cols = slice(q * 2 * BL, (q + 1) * 2 * BL)
                nc.sync.dma_start(xpb[:, cols], xp[:, cols])
            # scalar ring (parallel HWDGE ring): small consts + rkb
            bcast = cpool.tile([128, 4], F32, tag="bcast", name="bcast")
            nc.scalar.dma_start(bcast[:], gates[:])
            plb_sb = cpool.tile([128, 8], F32, tag="plb", name="plb")
            nc.scalar.dma_start(plb_sb[:], plbT[:])
            rkb = dpool.tile([128, NSLOT * SEQ], BF16, tag="rkb", name="rkb")
            for q in range(4):
                cols = slice(q * 4 * SEQ, (q + 1) * 4 * SEQ)
                nc.scalar.dma_start(rkb[:, cols], rkp[:, cols])

            shiftc = cpool.tile([128, 1], F32, tag="shiftc", name="shiftc")
            nc.vector.memset(shiftc[:], SHIFT)
            densb = cpool.tile([128, NCOL], F32, tag="densb", name="densb")
            nc.vector.memset(densb[:], 0.0)
            den_all = cpool.tile([128, NCOL], F32, tag="den_all", name="den_all")
            recip = cpool.tile([128, NCOL], F32, tag="recip", name="recip")
            cc_in = drpool.tile([128, NCOL], F32)
            cc_out = drpool.tile([128, NCOL], F32, addr_space="Shared")

            # ---------- persistent data tiles ----------
            EtP = [
                dpool.tile([128, 2 * BL], BF16, tag=f"EP{i}", name=f"EP{i}")
                for i in range(P1)
            ]
            Et2 = [
                dpool.tile([32, BL], BF16, tag=f"E2_{i}", name=f"E2_{i}")
                for i in range(P1)
            ]
            tP = [
                dpool.tile([128, 2 * BL], BF16, tag=f"tP{i}", name=f"tP{i}")
                for i in range(P1)
            ]
            t2 = [
                dpool.tile([32, BL], BF16, tag=f"t2_{i}", name=f"t2_{i}")
                for i in range(P1)
            ]
            ysbP = [
                dpool.tile([128, 2 * BL], BF16, tag=f"ysP{i}", name=f"ysP{i}")
                for i in range(P1)
            ]
            ysb2 = [
                dpool.tile([32, BL], BF16, tag=f"ys2_{i}", name=f"ys2_{i}")
                for i in range(P1)
            ]
            ypack = [
                dpool.tile([128, BL], BF16, tag=f"yp{k}", name=f"yp{k}")
                for k in range(2)
            ]

            # ---------- warm-up: dense PE burst so HAM un-throttles ----
            with (
                tc.tile_pool(name="wz", bufs=1) as wzpool,
                tc.tile_pool(name="psZ", bufs=1, space="PSUM") as psZ,
            ):
                zt = wzpool.tile([128, BL], BF16, tag="zt", name="zt")
                nc.vector.memset(zt[:], 0.0)
                pz = psZ.tile([128, BL], F32, tag="psz", name="psz")
                for _ in range(28):
                    nc.tensor.matmul(
                        pz[:], zt[0:128, 0:128], zt[:], start=True, stop=True
                    )

            # ---------- A1: logits + exp(+accum) ----------
            with (
                tc.tile_pool(name="psL", bufs=3, space="PSUM") as psL,
                tc.tile_pool(name="psLc", bufs=2, space="PSUM") as psLc,
            ):
                for i in range(P1):
                    pst = psL.tile([128, 2 * BL], F32, tag="psl", name=f"pst{i}")
                    pstc = psLc.tile([32, BL], F32, tag="pslc", name=f"pstc{i}")
                    for jt, (j0, jc) in enumerate(CH):
                        if jt < 2:
                            dst = pst[0:128, jt * BL : (jt + 1) * BL]
                        else:
                            dst = pstc[0:32, :]
                        for lt in range(3):
                            slot, off, cnt = xchunk(i, lt)
                            nc.tensor.matmul(
                                dst,
                                wpb[off : off + cnt, lt * N1 + j0 : lt * N1 + j0 + jc],
                                xpb[off : off + cnt, slot * BL : (slot + 1) * BL],
                                start=(lt == 0),
                                stop=(lt == 2),
                                tile_position=(off, 0) if off == 96 else None,
                            )
                    # one exp over both main chunks; den sums on DVE (keeps
                    # the ACT stream shorter than the PE stream per step)
                    nc.scalar.activation(
                        EtP[i][:], pst[0:128, :], AF.Exp,
                        bias=shiftc[0:128, 0:1], scale=SCALE,
                    )
                    nc.scalar.activation(
                        Et2[i][0:32, :], pstc[0:32, :], AF.Exp,
                        bias=shiftc[0:32, 0:1], scale=SCALE,
                    )
                    for jt in range(2):
                        nc.vector.tensor_reduce(
                            densb[0:128, i * 3 + jt : i * 3 + jt + 1],
                            EtP[i][0:128, jt * BL : (jt + 1) * BL],
                            mybir.AxisListType.X,
                            ALU.add,
                        )
                    nc.vector.tensor_reduce(
                        densb[0:32, i * 3 + 2 : i * 3 + 3],
                        Et2[i][0:32, :],
                        mybir.AxisListType.X,
                        ALU.add,
                    )

                # exp-sum AllReduce, triggered as soon as densb is complete
                nc.sync.dma_start(cc_in[:], densb[:])
                nc.gpsimd.collective_compute(
                    "AllReduce",
                    ALU.add,
                    replica_groups=[list(range(N_CORES))],
                    ins=[cc_in[:]],
                    outs=[cc_out[:]],
                )

            # ---------- A2: vT = (x_i @ wv)^T ; raw t = vT * E ----------
            with (
                tc.tile_pool(name="psV", bufs=2, space="PSUM") as psV,
                tc.tile_pool(name="psVc", bufs=2, space="PSUM") as psVc,
            ):
                for i in range(P1):
                    pv = psV.tile([128, 2 * BL], F32, tag="psv", name=f"pv{i}")
                    pvc = psVc.tile([32, BL], F32, tag="psvc", name=f"pvc{i}")
                    for nt, (n0, ncnt) in enumerate(CH):
                        if nt < 2:
                            dst = pv[0:128, nt * BL : (nt + 1) * BL]
                        else:
                            dst = pvc[0:32, :]
                        for lt in range(3):
                            slot, off, cnt = xchunk(i, lt)
                            nc.tensor.matmul(
                                dst,
                                wpb[
                                    off : off + cnt,
                                    (3 + lt) * N1 + n0 : (3 + lt) * N1 + n0 + ncnt,
                                ],
                                xpb[off : off + cnt, slot * BL : (slot + 1) * BL],
                                start=(lt == 0),
                                stop=(lt == 2),
                                tile_position=(off, 0) if off == 96 else None,
                            )
                    nc.vector.tensor_mul(tP[i][:], pv[0:128, :], EtP[i][:])
                    nc.vector.tensor_mul(
                        t2[i][0:32, :], pvc[0:32, :], Et2[i][0:32, :]
                    )
                # keep-warm fillers: bridge the PE idle window between the
                # last v-matmul and the first recip-gated projection matmul
                # so the HAM clock gate stays un-throttled into phase B
                pzb = psVc.tile([32, BL], F32, tag="psvc", name="pv_fill")
                for _ in range(10):
                    nc.tensor.matmul(
                        pzb[0:32, :], wpb[0:32, 0:32], xpb[0:32, 0:BL],
                        start=True, stop=True,
                    )

            nc.sync.dma_start(den_all[:], cc_out[:])
            nc.vector.reciprocal(recip[:], den_all[:])

            # ---------- B: gated scan (fused normalize) + projection ----
            with (
                tc.tile_pool(name="gat", bufs=3) as gpool,
                tc.tile_pool(name="osb", bufs=3) as opool,
                tc.tile_pool(name="psP", bufs=1, space="PSUM") as psP,
            ):
                pps = [
                    psP.tile([128, BL], F32, tag=f"pp{st}", name=f"pp{st}")
                    for st in range(8)
                ]
                proj_started = [False] * 8

                def proj(src_tile, coff, cnt, rk_slot, last):
                    # two concurrent col-tiled M=64 halves per (slot, st):
                    # each half's LDWEIGHTS overlaps the other half's
                    # rhs streaming (disjoint array column groups)
                    for st in range(8):
                        c0 = rk_slot * SEQ + st * 128
                        for h in range(2):
                            nc.tensor.matmul(
                                pps[st][64 * h : 64 * h + 64, :],
                                rkb[0:cnt, c0 + 64 * h : c0 + 64 * h + 64],
                                src_tile[0:cnt, coff : coff + BL],
                                start=not proj_started[st],
                                stop=last,
                                tile_position=(0, 64 * h),
                            )
                        proj_started[st] = True

                def scan_pair(i):
                    """ys_i = t_i * recip + tanh(..ys_{i-1}..)*sig(..)."""
                    if i == 0:
                        for c in range(2):
                            h = slice(c * BL, (c + 1) * BL)
                            nc.vector.tensor_scalar_mul(
                                ysbP[0][:, h], tP[0][:, h],
                                recip[0:128, c : c + 1],
                            )
                    else:
                        tt = gpool.tile([128, 2 * BL], BF16, tag="tt", name="tt")
                        ts = gpool.tile([128, 2 * BL], BF16, tag="ts", name="ts")
                        nc.scalar.activation(
                            tt[:], ysbP[i - 1][:], AF.Tanh,
                            bias=bcast[0:128, 2:3], scale=bcast[0:128, 0:1],
                        )
                        nc.scalar.activation(
                            ts[:], ysbP[i - 1][:], AF.Sigmoid,
                            bias=bcast[0:128, 3:4], scale=bcast[0:128, 1:2],
                        )
                        nc.vector.tensor_mul(tt[:], tt[:], ts[:])
                        for c in range(2):
                            h = slice(c * BL, (c + 1) * BL)
                            col = i * 3 + c
                            nc.vector.scalar_tensor_tensor(
                                ysbP[i][:, h], tP[i][:, h],
                                recip[0:128, col : col + 1], tt[:, h],
                                ALU.mult, ALU.add,
                            )

                def scan_slim(i):
                    seg = slice(0, 32)
                    col = i * 3 + 2
                    if i == 0:
                        nc.vector.tensor_scalar_mul(
                            ysb2[0][seg, :], t2[0][seg, :],
                            recip[seg, col : col + 1],
                        )
                    else:
                        tt = gpool.tile([32, BL], BF16, tag="tt2", name="tt2")
                        ts = gpool.tile([32, BL], BF16, tag="ts2", name="ts2")
                        nc.scalar.activation(
                            tt[seg, :], ysb2[i - 1][seg, :], AF.Tanh,
                            bias=bcast[seg, 2:3], scale=bcast[seg, 0:1],
                        )
                        nc.scalar.activation(
                            ts[seg, :], ysb2[i - 1][seg, :], AF.Sigmoid,
                            bias=bcast[seg, 3:4], scale=bcast[seg, 1:2],
                        )
                        nc.vector.tensor_mul(tt[seg, :], tt[seg, :], ts[seg, :])
                        nc.vector.scalar_tensor_tensor(
                            ysb2[i][seg, :], t2[i][seg, :],
                            recip[seg, col : col + 1], tt[seg, :],
                            ALU.mult, ALU.add,
                        )

                for i in range(P1):
                    scan_pair(i)
                    scan_slim(i)
                    for c in range(2):
                        proj(ysbP[i], c * BL, 128, 2 + 2 * i + c, False)
                    # repack the finished tail for the projection
                    pk, off = pack_pos(i)
                    nc.sync.dma_start(
                        ypack[pk][off : off + 32, :], ysb2[i][0:32, :]
                    )
                    if i == 3:
                        proj(ypack[0], 0, 128, 0, False)
                    elif i == 6:
                        proj(ypack[1], 0, 96, 1, True)

                # evacuate + bias into one contiguous bf16 tile (full-width
                # ops, ACT/DVE alternating); store in 4 chunks so the DMAs
                # overlap the remaining evacuations
                ob = opool.tile([128, 8 * BL], BF16, tag="ob", name="ob")
                for st in range(8):
                    cols = slice(st * BL, (st + 1) * BL)
                    if st % 2 == 0:
                        nc.scalar.activation(
                            ob[:, cols],
                            pps[st][:],
                            AF.Identity,
                            bias=plb_sb[:, st : st + 1],
                        )
                    else:
                        nc.vector.tensor_scalar_add(
                            ob[:, cols], pps[st][:], plb_sb[:, st : st + 1],
                        )
                        h = slice((st - 1) * BL, (st + 1) * BL)
                        nc.sync.dma_start(outT[:, h], ob[:, h])

    nc.compile()
    return nc


def _pack_feat(src, ncols):
    """[7, 288, ncols] (i, j, cols) -> packed [128, 16*ncols] bf16."""
    out = np.zeros((128, NSLOT, ncols), dtype=np.float32)
    for i in range(P1):
        out[:, 2 + 2 * i, :] = src[i, 0:128, :]
        out[:, 2 + 2 * i + 1, :] = src[i, 128:256, :]
    for k in range(4):
        out[32 * k : 32 * k + 32, 0, :] = src[k, 256:288, :]
    for k in range(3):
        out[32 * k : 32 * k + 32, 1, :] = src[4 + k, 256:288, :]
    return np.ascontiguousarray(out.reshape(128, NSLOT * ncols).astype(BF16NP))


def _pack_lhsT(W):
    """[288, 288] lhsT -> [128, 3*288] (chunks l 0:128 / 128:256 / tail x4)."""
    out = np.zeros((128, 3, N1), dtype=np.float32)
    out[:, 0, :] = W[0:128, :]
    out[:, 1, :] = W[128:256, :]
    for k in range(4):
        out[32 * k : 32 * k + 32, 2, :] = W[256:288, :]
    return out.reshape(128, 3 * N1)


def _prep_host(inputs):
    f32 = np.float32
    x = np.ascontiguousarray(inputs["x"], dtype=f32)
    plw = np.ascontiguousarray(inputs["proj_len_w"], dtype=f32)
    plb = np.ascontiguousarray(inputs["proj_len_b"], dtype=f32)
    h1 = np.ascontiguousarray(inputs["h1"], dtype=f32)
    wk = np.ascontiguousarray(inputs["w_k1"], dtype=f32)
    wv = np.ascontiguousarray(inputs["w_v1"], dtype=f32)

    # proj_len_w de-interleaved + transposed: [7, 288, 1024]
    plwT = plw.reshape(SEQ, N1, P1).transpose(2, 1, 0)
    rkp = _pack_feat(plwT, SEQ)

    # logits lhsT = wk @ h1.T (fused W_hk), computed in f32 on the host
    whk = wk @ h1.T
    wp = np.concatenate([_pack_lhsT(whk), _pack_lhsT(wv)], axis=1)
    wp = np.ascontiguousarray(wp.astype(BF16NP))

    plbT = np.ascontiguousarray(plb.reshape(8, 128).T)
    gates = np.tile(
        np.array(
            [[inputs["alpha1"][0], inputs["alpha2"][0],
              inputs["beta1"][0], inputs["beta2"][0]]],
            dtype=f32,
        ),
        (128, 1),
    )

    rep = {"rkp": rkp, "wp": wp, "plbT": plbT, "gates": gates}
    in_maps = []
    for c in range(N_CORES):
        xc = x[c * BL : (c + 1) * BL]  # [512, 2016]
        xT = xc.reshape(BL, N1, P1).transpose(2, 1, 0)  # [7, 288, 512]
        in_maps.append({"xp": _pack_feat(xT, BL), **rep})
    return in_maps


_NC = None


def _get_nc():
    global _NC
    if _NC is None:
        _NC = build()
    return _NC


def run(inputs, trace=False):
    nc = _get_nc()
    in_maps = _prep_host(inputs)
    res = run_bass_kernel_spmd(
        nc, in_maps, core_ids=list(range(N_CORES)), trace=trace
    )
    full = np.concatenate(
        [
            res.results[c]["outT"]
            .reshape(128, 8, BL)
            .transpose(2, 1, 0)
            .reshape(BL, SEQ)
            .astype(np.float32)
            for c in range(N_CORES)
        ],
        axis=0,
    )
    return np.ascontiguousarray(full), res


def kernel(**inputs):
    full, _ = run(inputs, trace=False)
    return full


# revision 32
# speedup vs baseline: 1.0518x; 1.0518x over previous
"""Trainium2 Bass kernel for nn_InternalMAFE_59270548684863.

Structure (final, trace-driven rewrite of the v2 baseline):
  - Only branch 1 (p=7, n=288) of the reference affects the output; the
    n2=1008 branch feeds a dead projection and is never computed.
  - Batch-sharded over 8 cores (512 rows each); softmax normalizes over
    the batch axis, so per-(step, feature) exp-sums are AllReduced.
    Constant-shift softmax exp(s*scale - 50) avoids a cross-core max.
  - Changes vs the v2 baseline:
      * All big tensors are cast to bf16 HOST-side and loaded with plain
        HWDGE DMA (the v2 SWDGE cast-DMA path ran ~3x below line rate);
        the logits weights + first x slots load first so A1 starts ASAP.
      * W_hk = w_k1 @ h1.T is precomputed on the host (removes the
        on-device PE-transpose prep phase).
      * A PE warm-up burst runs at t=0 and the matmul stream is kept
        dense (all logits -> all v -> projection; psL triple-buffered)
        to keep the HAM clock gate at high clock.
      * One combined exp-sum AllReduce [128,21] triggered as soon as the
        den sums retire (exp pair on ACT, den column sums on DVE). The
        ncfw path costs barrier(20-60us)+wakeup(~12us)+mesh(10-28us);
        a raw remote-DMA butterfly was tried and is schedulable via two
        TileContexts with a raw DVE wait between them, but loses: the
        peers' data arrives later than the ncfw protocol completes here.
      * Softmax normalization is fused into the gated scan with one
        scalar_tensor_tensor: ys_i = (t_i * recip) + tanh(..)*sig(..).
      * Output is stored bf16 in one contiguous packed tile (full-width
        evacuations, 4 overlapped store DMAs); host re-packs + up-casts.
"""

import math

import numpy as np
import ml_dtypes

import concourse.bacc as bacc
import concourse.mybir as mybir
import concourse.tile as tile
from concourse.bass_utils import run_bass_kernel_spmd

N_CORES = 8
B = 4096
BL = B // N_CORES  # 512 rows per core
INP = 2016
P1 = 7
N1 = 288
SEQ = 1024
SCALE = 1.0 / math.sqrt(N1)
SHIFT = -50.0
F32 = mybir.dt.float32
BF16 = mybir.dt.bfloat16
AF = mybir.ActivationFunctionType
ALU = mybir.AluOpType
BF16NP = ml_dtypes.bfloat16

NSLOT = 16
CH = [(0, 128), (128, 128), (256, 32)]


def xchunk(i, lt):
    """K-side: (slot, partition offset, count) of x chunk lt of step i."""
    if lt < 2:
        return 2 + 2 * i + lt, 0, 128
    if i < 4:
        return 0, 32 * i, 32
    return 1, 32 * (i - 4), 32


def pack_pos(i):
    """(pack tile index, partition offset) of step i's 32-row tail."""
    if i < 4:
        return 0, 32 * i
    return 1, 32 * (i - 4)


def build():
    nc = bacc.Bacc(
        "TRN2", target_bir_lowering=False, debug=False, num_devices=N_CORES
    )
    # all big inputs pre-cast to bf16 on the host
    xp = nc.dram_tensor("xp", [128, NSLOT * BL], BF16, kind="ExternalInput").ap()
    rkp = nc.dram_tensor("rkp", [128, NSLOT * SEQ], BF16, kind="ExternalInput").ap()
    # wp: [whk lt0|lt1|lt2] [wv lt0|lt1|lt2], each [128, 288]
    wp = nc.dram_tensor("wp", [128, 6 * N1], BF16, kind="ExternalInput").ap()
    plbT = nc.dram_tensor("plbT", [128, 8], F32, kind="ExternalInput").ap()
    gates = nc.dram_tensor("gates", [128, 4], F32, kind="ExternalInput").ap()
    # outT[p, st*BL + b] holds out[b, st*128 + p] (host re-packs)
    outT = nc.dram_tensor("outT", [128, 8 * BL], BF16, kind="ExternalOutput").ap()

    NCOL = 3 * P1  # 21 den columns

    with tile.TileContext(nc) as tc:
        with (
            tc.tile_pool(name="const", bufs=1) as cpool,
            tc.tile_pool(name="data", bufs=1) as dpool,
            tc.tile_pool(name="dram", bufs=1, space="DRAM") as drpool,
        ):
            # ---------- loads (plain HWDGE, bf16) ----------
            # sync ring: logits weights first, then x slots in consumption
            # order (v weights can land later; A2 runs after A1)
            wpb = dpool.tile([128, 6 * N1], BF16, tag="wpb", name="wpb")
            nc.sync.dma_start(wpb[:, 0 : 3 * N1], wp[:, 0 : 3 * N1])
            xpb = dpool.tile([128, NSLOT * BL], BF16, tag="xpb", name="xpb")
            for q in (1, 0, 2):
                cols = slice(q * 2 * BL, (q + 1) * 2 * BL)
                nc.sync.dma_start(xpb[:, cols], xp[:, cols])
            nc.sync.dma_start(wpb[:, 3 * N1 : 6 * N1], wp[:, 3 * N1 : 6 * N1])
            for q in (3, 4, 5, 6, 7):
                cols = slice(q * 2 * BL, (q + 1) * 2 * BL)
                nc.sync.dma_start(xpb[:, cols], xp[:, cols])
            # scalar ring (parallel HWDGE ring): small consts + rkb
            bcast = cpool.tile([128, 4], F32, tag="bcast", name="bcast")
            nc.scalar.dma_start(bcast[:], gates[:])
            plb_sb = cpool.tile([128, 8], F32, tag="plb", name="plb")
            nc.scalar.dma_start(plb_sb[:], plbT[:])
            rkb = dpool.tile([128, NSLOT * SEQ], BF16, tag="rkb", name="rkb")
            for q in range(4):
                cols = slice(q * 4 * SEQ, (q + 1) * 4 * SEQ)
                nc.scalar.dma_start(rkb[:, cols], rkp[:, cols])

            shiftc = cpool.tile([128, 1], F32, tag="shiftc", name="shiftc")
            nc.vector.memset(shiftc[:], SHIFT)
            densb = cpool.tile([128, NCOL], F32, tag="densb", name="densb")
            nc.vector.memset(densb[:], 0.0)
            den_all = cpool.tile([128, NCOL], F32, tag="den_all", name="den_all")
            recip = cpool.tile([128, NCOL], F32, tag="recip", name="recip")
            cc_in = drpool.tile([128, NCOL], F32)
            cc_out = drpool.tile([128, NCOL], F32, addr_space="Shared")

            # ---------- persistent data tiles ----------
            EtP = [
                dpool.tile([128, 2 * BL], BF16, tag=f"EP{i}", name=f"EP{i}")
                for i in range(P1)
            ]
            Et2 = [
                dpool.tile([32, BL], BF16, tag=f"E2_{i}", name=f"E2_{i}")
                for i in range(P1)
            ]
            tP = [
                dpool.tile([128, 2 * BL], BF16, tag=f"tP{i}", name=f"tP{i}")
                for i in range(P1)
            ]
            t2 = [
                dpool.tile([32, BL], BF16, tag=f"t2_{i}", name=f"t2_{i}")
                for i in range(P1)
            ]
            ysbP = [
                dpool.tile([128, 2 * BL], BF16, tag=f"ysP{i}", name=f"ysP{i}")
                for i in range(P1)
            ]
            ysb2 = [
                dpool.tile([32, BL], BF16, tag=f"ys2_{i}", name=f"ys2_{i}")
                for i in range(P1)
            ]
            ypack = [
                dpool.tile([128, BL], BF16, tag=f"yp{k}", name=f"yp{k}")
                for k in range(2)
            ]

            # ---------- warm-up: dense PE burst so HAM un-throttles ----
            with (
                tc.tile_pool(name="wz", bufs=1) as wzpool,
                tc.tile_pool(name="psZ", bufs=1, space="PSUM") as psZ,
            ):
                zt = wzpool.tile([128, BL], BF16, tag="zt", name="zt")
                nc.vector.memset(zt[:], 0.0)
                pz = psZ.tile([128, BL], F32, tag="psz", name="psz")
                for _ in range(28):
                    nc.tensor.matmul(
                        pz[:], zt[0:128, 0:128], zt[:], start=True, stop=True
                    )

            # ---------- A1: logits + exp(+accum) ----------
            with (
                tc.tile_pool(name="psL", bufs=3, space="PSUM") as psL,
                tc.tile_pool(name="psLc", bufs=2, space="PSUM") as psLc,
            ):
                for i in range(P1):
                    pst = psL.tile([128, 2 * BL], F32, tag="psl", name=f"pst{i}")
                    pstc = psLc.tile([32, BL], F32, tag="pslc", name=f"pstc{i}")
                    for jt, (j0, jc) in enumerate(CH):
                        if jt < 2:
                            dst = pst[0:128, jt * BL : (jt + 1) * BL]
                        else:
                            dst = pstc[0:32, :]
                        for lt in range(3):
                            slot, off, cnt = xchunk(i, lt)
                            nc.tensor.matmul(
                                dst,
                                wpb[off : off + cnt, lt * N1 + j0 : lt * N1 + j0 + jc],
                                xpb[off : off + cnt, slot * BL : (slot + 1) * BL],
                                start=(lt == 0),
                                stop=(lt == 2),
                                tile_position=(off, 0) if off == 96 else None,
                            )
                    # one exp over both main chunks; den sums on DVE (keeps
                    # the ACT stream shorter than the PE stream per step)
                    nc.scalar.activation(
                        EtP[i][:], pst[0:128, :], AF.Exp,
                        bias=shiftc[0:128, 0:1], scale=SCALE,
                    )
                    nc.scalar.activation(
                        Et2[i][0:32, :], pstc[0:32, :], AF.Exp,
                        bias=shiftc[0:32, 0:1], scale=SCALE,
                    )
                    for jt in range(2):
                        nc.vector.tensor_reduce(
                            densb[0:128, i * 3 + jt : i * 3 + jt + 1],
                            EtP[i][0:128, jt * BL : (jt + 1) * BL],
                            mybir.AxisListType.X,
                            ALU.add,
                        )
                    nc.vector.tensor_reduce(
                        densb[0:32, i * 3 + 2 : i * 3 + 3],
                        Et2[i][0:32, :],
                        mybir.AxisListType.X,
                        ALU.add,
                    )

                # exp-sum AllReduce, triggered as soon as densb is complete
                nc.sync.dma_start(cc_in[:], densb[:])
                nc.gpsimd.collective_compute(
                    "AllReduce",
                    ALU.add,
                    replica_groups=[list(range(N_CORES))],
                    ins=[cc_in[:]],
                    outs=[cc_out[:]],
                )

            # ---------- A2: vT = (x_i @ wv)^T ; raw t = vT * E ----------
            with (
                tc.tile_pool(name="psV", bufs=2, space="PSUM") as psV,
                tc.tile_pool(name="psVc", bufs=2, space="PSUM") as psVc,
            ):
                for i in range(P1):
                    pv = psV.tile([128, 2 * BL], F32, tag="psv", name=f"pv{i}")
                    pvc = psVc.tile([32, BL], F32, tag="psvc", name=f"pvc{i}")
                    for nt, (n0, ncnt) in enumerate(CH):
                        if nt < 2:
                            dst = pv[0:128, nt * BL : (nt + 1) * BL]
                        else:
                            dst = pvc[0:32, :]
                        for lt in range(3):
                            slot, off, cnt = xchunk(i, lt)
                            nc.tensor.matmul(
                                dst,
                                wpb[
                                    off : off + cnt,
                                    (3 + lt) * N1 + n0 : (3 + lt) * N1 + n0 + ncnt,
                                ],
                                xpb[off : off + cnt, slot * BL : (slot + 1) * BL],
                                start=(lt == 0),
                                stop=(lt == 2),
                                tile_position=(off, 0) if off == 96 else None,
                            )
                    nc.vector.tensor_mul(tP[i][:], pv[0:128, :], EtP[i][:])
                    nc.vector.tensor_mul(
                        t2[i][0:32, :], pvc[0:32, :], Et2[i][0:32, :]
                    )
                # keep-warm fillers: bridge the PE idle window between the
                # last v-matmul and the first recip-gated projection matmul
                # so the HAM clock gate stays un-throttled into phase B
                pzb = psVc.tile([32, BL], F32, tag="psvc", name="pv_fill")
                for _ in range(10):
                    nc.tensor.matmul(
                        pzb[0:32, :], wpb[0:32, 0:32], xpb[0:32, 0:BL],
                        start=True, stop=True,
                    )

            nc.sync.dma_start(den_all[:], cc_out[:])
            nc.vector.reciprocal(recip[:], den_all[:])

            # ---------- B: gated scan (fused normalize) + projection ----
            with (
                tc.tile_pool(name="gat", bufs=3) as gpool,
                tc.tile_pool(name="osb", bufs=3) as opool,
                tc.tile_pool(name="psP", bufs=1, space="PSUM") as psP,
            ):
                pps = [
                    psP.tile([128, BL], F32, tag=f"pp{st}", name=f"pp{st}")
                    for st in range(8)
                ]
                proj_started = [False] * 8

                def proj(src_tile, coff, cnt, rk_slot, last):
                    # two concurrent col-tiled M=64 halves per (slot, st):
                    # each half's LDWEIGHTS overlaps the other half's
                    # rhs streaming (disjoint array column groups)
                    for st in range(8):
                        c0 = rk_slot * SEQ + st * 128
                        for h in range(2):
                            nc.tensor.matmul(
                                pps[st][64 * h : 64 * h + 64, :],
                                rkb[0:cnt, c0 + 64 * h : c0 + 64 * h + 64],
                                src_tile[0:cnt, coff : coff + BL],
                                start=not proj_started[st],
                                stop=last,
                                tile_position=(0, 64 * h),
                            )
                        proj_started[st] = True

                def scan_pair(i):
                    """ys_i = t_i * recip + tanh(..ys_{i-1}..)*sig(..)."""
                    if i == 0:
                        for c in range(2):
                            h = slice(c * BL, (c + 1) * BL)
                            nc.vector.tensor_scalar_mul(
                                ysbP[0][:, h], tP[0][:, h],
                                recip[0:128, c : c + 1],
                            )
                    else:
                        tt = gpool.tile([128, 2 * BL], BF16, tag="tt", name="tt")
                        ts = gpool.tile([128, 2 * BL], BF16, tag="ts", name="ts")
                        nc.scalar.activation(
                            tt[:], ysbP[i - 1][:], AF.Tanh,
                            bias=bcast[0:128, 2:3], scale=bcast[0:128, 0:1],
                        )
                        nc.scalar.activation(
                            ts[:], ysbP[i - 1][:], AF.Sigmoid,
                            bias=bcast[0:128, 3:4], scale=bcast[0:128, 1:2],
                        )
                        nc.vector.tensor_mul(tt[:], tt[:], ts[:])
                        for c in range(2):
                            h = slice(c * BL, (c + 1) * BL)
                            col = i * 3 + c
                            nc.vector.scalar_tensor_tensor(
                                ysbP[i][:, h], tP[i][:, h],
                                recip[0:128, col : col + 1], tt[:, h],
                                ALU.mult, ALU.add,
                            )

                def scan_slim(i):
                    seg = slice(0, 32)
                    col = i * 3 + 2
                    if i == 0:
                        nc.vector.tensor_scalar_mul(
                            ysb2[0][seg, :], t2[0][seg, :],
                            recip[seg, col : col + 1],
                        )
                    else:
                        tt = gpool.tile([32, BL], BF16, tag="tt2", name="tt2")
                        ts = gpool.tile([32, BL], BF16, tag="ts2", name="ts2")
                        nc.scalar.activation(
                            tt[seg, :], ysb2[i - 1][seg, :], AF.Tanh,
                            bias=bcast[seg, 2:3], scale=bcast[seg, 0:1],
                        )
                        nc.scalar.activation(
                            ts[seg, :], ysb2[i - 1][seg, :], AF.Sigmoid,
                            bias=bcast[seg, 3:4], scale=bcast[seg, 1:2],
                        )
                        nc.vector.tensor_mul(tt[seg, :], tt[seg, :], ts[seg, :])
                        nc.vector.scalar_tensor_tensor(
                            ysb2[i][seg, :], t2[i][seg, :],
                            recip[seg, col : col + 1], tt[seg, :],
                            ALU.mult, ALU.add,
                        )

                for i in range(P1):
                    scan_pair(i)
                    scan_slim(i)
                    for c in range(2):
                        proj(ysbP[i], c * BL, 128, 2 + 2 * i + c, False)
                    # repack the finished tail for the projection
                    pk, off = pack_pos(i)
                    nc.sync.dma_start(
                        ypack[pk][off : off + 32, :], ysb2[i][0:32, :]
                    )
                    if i == 3:
                        proj(ypack[0], 0, 128, 0, False)
                    elif i == 6:
                        proj(ypack[1], 0, 96, 1, True)

                # evacuate + bias into one contiguous bf16 tile (full-width
                # ops, ACT/DVE alternating); store in 4 chunks so the DMAs
                # overlap the remaining evacuations
                ob = opool.tile([128, 8 * BL], BF16, tag="ob", name="ob")
                for st in range(8):
                    cols = slice(st * BL, (st + 1) * BL)
                    if st % 2 == 0:
                        nc.scalar.activation(
                            ob[:, cols],
                            pps[st][:],
                            AF.Identity,
                            bias=plb_sb[:, st : st + 1],
                        )
                    else:
                        nc.vector.tensor_scalar_add(
                            ob[:, cols], pps[st][:], plb_sb[:, st : st + 1],
                        )
                        h = slice((st - 1) * BL, (st + 1) * BL)
                        nc.sync.dma_start(outT[:, h], ob[:, h])

    nc.compile()
    return nc


def _pack_feat(src, ncols):
    """[7, 288, ncols] (i, j, cols) -> packed [128, 16*ncols] bf16."""
    out = np.zeros((128, NSLOT, ncols), dtype=np.float32)
    for i in range(P1):
        out[:, 2 + 2 * i, :] = src[i, 0:128, :]
        out[:, 2 + 2 * i + 1, :] = src[i, 128:256, :]
    for k in range(4):
        out[32 * k : 32 * k + 32, 0, :] = src[k, 256:288, :]
    for k in range(3):
        out[32 * k : 32 * k + 32, 1, :] = src[4 + k, 256:288, :]
    return np.ascontiguousarray(out.reshape(128, NSLOT * ncols).astype(BF16NP))


def _pack_lhsT(W):
    """[288, 288] lhsT -> [128, 3*288] (chunks l 0:128 / 128:256 / tail x4)."""
    out = np.zeros((128, 3, N1), dtype=np.float32)
    out[:, 0, :] = W[0:128, :]
    out[:, 1, :] = W[128:256, :]
    for k in range(4):
        out[32 * k : 32 * k + 32, 2, :] = W[256:288, :]
    return out.reshape(128, 3 * N1)


def _prep_host(inputs):
    f32 = np.float32
    x = np.ascontiguousarray(inputs["x"], dtype=f32)
    plw = np.ascontiguousarray(inputs["proj_len_w"], dtype=f32)
    plb = np.ascontiguousarray(inputs["proj_len_b"], dtype=f32)
    h1 = np.ascontiguousarray(inputs["h1"], dtype=f32)
    wk = np.ascontiguousarray(inputs["w_k1"], dtype=f32)
    wv = np.ascontiguousarray(inputs["w_v1"], dtype=f32)

    # proj_len_w de-interleaved + transposed: [7, 288, 1024]
    plwT = plw.reshape(SEQ, N1, P1).transpose(2, 1, 0)
    rkp = _pack_feat(plwT, SEQ)

    # logits lhsT = wk @ h1.T (fused W_hk), computed in f32 on the host
    whk = wk @ h1.T
    wp = np.concatenate([_pack_lhsT(whk), _pack_lhsT(wv)], axis=1)
    wp = np.ascontiguousarray(wp.astype(BF16NP))

    plbT = np.ascontiguousarray(plb.reshape(8, 128).T)
    gates = np.tile(
        np.array(
            [[inputs["alpha1"][0], inputs["alpha2"][0],
              inputs["beta1"][0], inputs["beta2"][0]]],
            dtype=f32,
        ),
        (128, 1),
    )

    rep = {"rkp": rkp, "wp": wp, "plbT": plbT, "gates": gates}
    in_maps = []
    for c in range(N_CORES):
        xc = x[c * BL : (c + 1) * BL]  # [512, 2016]
        xT = xc.reshape(BL, N1, P1).transpose(2, 1, 0)  # [7, 288, 512]
        in_maps.append({"xp": _pack_feat(xT, BL), **rep})
    return in_maps


_NC = None


def _get_nc():
    global _NC
    if _NC is None:
        _NC = build()
    return _NC


def run(inputs, trace=False):
    nc = _get_nc()
    in_maps = _prep_host(inputs)
    res = run_bass_kernel_spmd(
        nc, in_maps, core_ids=list(range(N_CORES)), trace=trace
    )
    full = np.concatenate(
        [
            res.results[c]["outT"]
            .reshape(128, 8, BL)
            .transpose(2, 1, 0)
            .reshape(BL, SEQ)
            .astype(np.float32)
            for c in range(N_CORES)
        ],
        axis=0,
    )
    return np.ascontiguousarray(full), res


def kernel(**inputs):
    full, _ = run(inputs, trace=False)
    return full
